# revision 24
# baseline (speedup 1.0000x reference)
"""Trainium2 Bass kernel for additive (Bahdanau-style) attention.

Computation (per batch row b):
    energy  = tanh((enc[b] + hidden[b]) @ W^T + b_attn)   # [S, H]
    scores  = energy @ v                                  # [S]
    attn    = softmax(scores)                             # [S]
    context = attn @ enc[b]                               # [H]
Returns (context [B,H], attn_weights [B,S]).

Strategy: data-parallel over batch across 8 NeuronCores (4 batches/core).
The big GEMM (enc @ W^T, 137 GFLOP total) runs on the TensorEngine in
float32r (full-rate fp32 streaming mode). The contraction over H requires
H on SBUF partitions, so the host pre-transposes encoder_outputs once
([B,S,H] -> [B,H,S]); the context matmul contracts over S and uses the
natural layout. The hidden@W^T+b term is folded into a per-(batch,o)
bias column applied inside the fused tanh activation.
"""

import os
import sys

sys.path.insert(0, "/opt/trn_rl_repo")

import numpy as np

import concourse.bass as bass  # noqa: F401  (registers engine types)
import concourse.mybir as mybir
from concourse import bacc
from concourse.bass_utils import run_bass_kernel_spmd
from concourse.masks import make_identity
from concourse.tile import TileContext

B, S, H = 32, 2048, 1024
NCORES = 8
BL = B // NCORES          # batches per core
HC = H // 128             # 8 contraction chunks
OT = H // 128             # 8 output-feature tiles
NT = 512                  # matmul moving free dim
ST = S // NT              # 4 s-tiles per batch
SC = S // 128             # 16 s-chunks per batch (context contraction)

F32 = mybir.dt.float32
F32R = mybir.dt.float32r
BF16 = mybir.dt.bfloat16
AXX = mybir.AxisListType.X
TANH = mybir.ActivationFunctionType.Tanh
EXP = mybir.ActivationFunctionType.Exp

_compiled_nc = None


def _build():
    nc = bacc.Bacc("TRN2", target_bir_lowering=False, debug=False, num_devices=NCORES)

    encT_d = nc.dram_tensor("encT", [BL, H, S], F32R, kind="ExternalInput")
    enc_d = nc.dram_tensor("enc", [BL, S, H], F32, kind="ExternalInput")
    wt_d = nc.dram_tensor("wt", [H, H], F32, kind="ExternalInput")       # W^T [h, o]
    hid_d = nc.dram_tensor("hid", [128, HC, BL], F32, kind="ExternalInput")
    vc_d = nc.dram_tensor("vc", [128, OT], F32, kind="ExternalInput")
    bc_d = nc.dram_tensor("bc", [128, OT], F32, kind="ExternalInput")
    ctx_d = nc.dram_tensor("ctx", [BL, H], F32, kind="ExternalOutput")
    attn_d = nc.dram_tensor("attn", [BL, S], F32, kind="ExternalOutput")

    with TileContext(nc) as tc:
        with (
            tc.tile_pool(name="singles", bufs=1) as singles,
            tc.tile_pool(name="xT", bufs=2) as xpool,
            tc.tile_pool(name="energy", bufs=3) as epool,
            tc.tile_pool(name="encn", bufs=12) as encpool,
            tc.tile_pool(name="smalls", bufs=4) as smalls,
            tc.tile_pool(name="attnp", bufs=2) as attnp,
            tc.tile_pool(name="acols", bufs=2) as acols,
            tc.tile_pool(name="ctxp", bufs=2) as ctxp,
            tc.tile_pool(name="ps_e", bufs=2, space="PSUM") as ps_e,
            tc.tile_pool(name="ps_sc", bufs=2, space="PSUM") as ps_sc,
            tc.tile_pool(name="ps_c", bufs=1, space="PSUM") as ps_c,
            tc.tile_pool(name="ps_b", bufs=1, space="PSUM") as ps_b,
            tc.tile_pool(name="ps_t", bufs=1, space="PSUM") as ps_t_pool,
        ):
            # ---- resident parameters -------------------------------------
            # Small params first (the bias matmuls need them immediately);
            # W^T per h-chunk so the first matmuls start a couple of us in
            # instead of waiting for the whole 4MB.
            hid_sb = singles.tile([128, HC, BL], F32R)
            nc.gpsimd.dma_start(out=hid_sb[:], in_=hid_d.ap())
            vc_sb = singles.tile([128, OT], F32R)
            nc.gpsimd.dma_start(out=vc_sb[:], in_=vc_d.ap())
            bc_sb = singles.tile([128, OT], F32)
            nc.sync.dma_start(out=bc_sb[:], in_=bc_d.ap())
            ident16 = singles.tile([SC, SC], F32)
            make_identity(nc, ident16[:])
            wt_sb = singles.tile([128, HC, H], F32R)     # [h_local, hc, o]
            wt_r = wt_d.ap().rearrange("(hc p) o -> hc p o", p=128)
            for hc in range(HC):
                nc.gpsimd.dma_start(out=wt_sb[:, hc, :], in_=wt_r[hc])

            # ---- per-(batch, o) bias: hidden @ W^T + b_attn --------------
            # Computed transposed ([batch, o], 16 wide matmuls) then flipped
            # with 8 small PE transposes — much cheaper than 64 narrow
            # matmuls, and consumes W^T chunks as their DMAs land.
            bias_sb = singles.tile([128, OT, BL], F32)
            biasT_sb = singles.tile([BL, H], F32)
            for half in range(2):
                bps = ps_b.tile([BL, NT], F32)
                for hc in range(HC):
                    nc.tensor.matmul(
                        bps[:],
                        hid_sb[:, hc, :],
                        wt_sb[:, hc, half * NT : (half + 1) * NT],
                        start=(hc == 0),
                        stop=(hc == HC - 1),
                    )
                nc.vector.tensor_copy(
                    out=biasT_sb[:, half * NT : (half + 1) * NT], in_=bps[:]
                )
            for ot in range(OT):
                tps = ps_t_pool.tile([128, BL], F32, tag="ps_t")
                nc.tensor.transpose(
                    out=tps[:],
                    in_=biasT_sb[:, ot * 128 : (ot + 1) * 128],
                    identity=ident16[0:BL, 0:BL],
                )
                nc.vector.tensor_scalar_add(bias_sb[:, ot, :], tps[:], bc_sb[:, ot : ot + 1])

            def emit_context(b, st16):
                """context[b] = attn[b] @ enc[b], contracting over S.

                Deferred by one batch so the in-order PE stream never stalls
                on batch b's softmax: by the time PE reaches these
                instructions it has already run batch b+1's GEMM.
                """
                ps_t = ps_t_pool.tile([128, SC], F32, tag="ps_t")
                nc.tensor.transpose(out=ps_t[:], in_=st16[:], identity=ident16[:])
                atc = acols.tile([128, SC], BF16)
                nc.vector.tensor_copy(out=atc[:], in_=ps_t[:])
                psc = ps_c.tile([1, H], F32)
                for sc in range(SC):
                    encn = encpool.tile([128, H], BF16)
                    nc.gpsimd.dma_start(
                        out=encn[:], in_=enc_d.ap()[b, sc * 128 : (sc + 1) * 128, :]
                    )
                    nc.tensor.matmul(
                        psc[:, 0:NT], atc[:, sc : sc + 1], encn[:, 0:NT],
                        start=(sc == 0), stop=(sc == SC - 1),
                    )
                    nc.tensor.matmul(
                        psc[:, NT:H], atc[:, sc : sc + 1], encn[:, NT:H],
                        start=(sc == 0), stop=(sc == SC - 1),
                    )
                ctxs = ctxp.tile([1, H], F32)
                nc.vector.tensor_copy(out=ctxs[:], in_=psc[:])
                nc.sync.dma_start(out=ctx_d.ap()[b : b + 1, :], in_=ctxs[:])

            prev = None  # (batch_idx, atc tile) whose context emission is deferred
            for b in range(BL):
                # ---- energy GEMM + v-reduction -> scores -----------------
                # Each v-matmul is deferred by one ot-group (across s_tile
                # boundaries too) so the tanh it consumes finishes in the
                # shadow of the next group's 8 GEMM matmuls and the in-order
                # PE stream never stalls.
                scores_sb = attnp.tile([1, S], F32)
                pending_v = None   # ((args), (kwargs), scores_copy_or_None)
                pss_tiles = {}

                def flush_v():
                    nonlocal pending_v
                    if pending_v is not None:
                        args, kwargs, copy_args = pending_v
                        nc.tensor.matmul(*args, **kwargs)
                        if copy_args is not None:
                            nc.vector.tensor_copy(out=copy_args[0], in_=copy_args[1])
                        pending_v = None

                for st in range(ST):
                    xT = xpool.tile([128, HC, NT], F32R)
                    nc.sync.dma_start(
                        out=xT[:],
                        in_=encT_d.ap()[b].rearrange("(hc p) s -> p hc s", p=128)[
                            :, :, st * NT : (st + 1) * NT
                        ],
                    )
                    pss = ps_sc.tile([1, NT], F32)
                    pss_tiles[st] = pss
                    for ot in range(OT):
                        pse = ps_e.tile([128, NT], F32)
                        for hc in range(HC):
                            nc.tensor.matmul(
                                pse[:],
                                wt_sb[:, hc, ot * 128 : (ot + 1) * 128],
                                xT[:, hc, :],
                                start=(hc == 0),
                                stop=(hc == HC - 1),
                            )
                        en = epool.tile([128, NT], F32R)
                        nc.scalar.activation(
                            out=en[:], in_=pse[:], func=TANH,
                            bias=bias_sb[:, ot, b : b + 1], scale=1.0,
                        )
                        flush_v()
                        copy_args = None
                        if ot == OT - 1:
                            copy_args = (
                                scores_sb[:, st * NT : (st + 1) * NT],
                                pss[:],
                            )
                        pending_v = (
                            (pss[:], vc_sb[:, ot : ot + 1], en[:]),
                            dict(start=(ot == 0), stop=(ot == OT - 1)),
                            copy_args,
                        )
                flush_v()

                # ---- softmax over S (single-lane, in place) --------------
                negmax = smalls.tile([1, 1], F32)
                nc.vector.reduce_max(negmax[:], scores_sb[:], axis=AXX, negate=True)
                nc.scalar.activation(
                    out=scores_sb[:], in_=scores_sb[:], func=EXP, bias=negmax[:], scale=1.0
                )
                ssum = smalls.tile([1, 1], F32)
                nc.vector.reduce_sum(ssum[:], scores_sb[:], axis=AXX)
                rinv = smalls.tile([1, 1], F32)
                nc.vector.reciprocal(rinv[:], ssum[:])
                nc.vector.tensor_scalar_mul(scores_sb[:], scores_sb[:], rinv[:])
                nc.sync.dma_start(out=attn_d.ap()[b : b + 1, :], in_=scores_sb[:])

                # attn row reshaped to [16, 128] with one contiguous DMA; the
                # PE transpose to [128, 16] columns happens in emit_context.
                st16 = acols.tile([SC, 128], F32, tag="st16")
                nc.gpsimd.dma_start(out=st16[:], in_=scores_sb[0:1, :])

                if prev is not None:
                    emit_context(*prev)
                prev = (b, st16)

            emit_context(*prev)

    nc.compile()
    return nc


def _get_nc():
    global _compiled_nc
    if _compiled_nc is None:
        _compiled_nc = _build()
    return _compiled_nc


def kernel(hidden, encoder_outputs, W_attn, b_attn, v, _want_results=False, **run_kwargs):
    hidden = np.asarray(hidden, dtype=np.float32)
    enc = np.asarray(encoder_outputs, dtype=np.float32)
    W_attn = np.asarray(W_attn, dtype=np.float32)
    b_attn = np.asarray(b_attn, dtype=np.float32)
    v = np.asarray(v, dtype=np.float32)

    WT = np.ascontiguousarray(W_attn.T)                       # [h, o]
    vc = np.ascontiguousarray(v.reshape(OT, 128).T)           # [128, ot]
    bc = np.ascontiguousarray(b_attn.reshape(OT, 128).T)      # [128, ot]

    in_maps = []
    for i in range(NCORES):
        sl = slice(i * BL, (i + 1) * BL)
        enc_i = np.ascontiguousarray(enc[sl])                 # [BL, S, H]
        encT_i = np.ascontiguousarray(enc[sl].transpose(0, 2, 1))  # [BL, H, S]
        hid_i = np.ascontiguousarray(
            hidden[sl].reshape(BL, HC, 128).transpose(2, 1, 0)
        )                                                      # [128, hc, b]
        in_maps.append(
            {"encT": encT_i, "enc": enc_i, "wt": WT, "hid": hid_i, "vc": vc, "bc": bc}
        )

    nc = _get_nc()
    res = run_bass_kernel_spmd(nc, in_maps, core_ids=list(range(NCORES)), **run_kwargs)
    context = np.concatenate([res.results[i]["ctx"] for i in range(NCORES)], axis=0)
    attn = np.concatenate([res.results[i]["attn"] for i in range(NCORES)], axis=0)
    if _want_results:
        return (context, attn), res
    return (context, attn)


if __name__ == "__main__":
    rng = np.random.default_rng(0)
    out = kernel(
        hidden=rng.standard_normal((B, H), dtype=np.float32),
        encoder_outputs=rng.standard_normal((B, S, H), dtype=np.float32),
        W_attn=rng.standard_normal((H, H), dtype=np.float32) / 32.0,
        b_attn=rng.standard_normal(H, dtype=np.float32) * 0.01,
        v=rng.random(H, dtype=np.float32),
    )
    print("context", out[0].shape, "attn", out[1].shape)


# revision 30
# speedup vs baseline: 1.0819x; 1.0819x over previous
"""Trainium2 Bass kernel for additive (Bahdanau-style) attention.

Computation (per batch row b):
    energy  = tanh((enc[b] + hidden[b]) @ W^T + b_attn)   # [S, H]
    scores  = energy @ v                                  # [S]
    attn    = softmax(scores)                             # [S]
    context = attn @ enc[b]                               # [H]
Returns (context [B,H], attn_weights [B,S]).

Strategy: data-parallel over batch across 8 NeuronCores (4 batches/core).
The big GEMM (enc @ W^T, 137 GFLOP total) runs on the TensorEngine in
float32r (full-rate fp32 streaming mode). The contraction over H requires
H on SBUF partitions, so the host pre-transposes encoder_outputs once
([B,S,H] -> [B,H,S]); the context matmul contracts over S and uses the
natural layout. The hidden@W^T+b term is folded into a per-(batch,o)
bias column applied inside the fused tanh activation.
"""

import os
import sys

sys.path.insert(0, "/opt/trn_rl_repo")

import ml_dtypes
import numpy as np

import concourse.bass as bass  # noqa: F401  (registers engine types)
import concourse.mybir as mybir
from concourse import bacc
from concourse.bass_utils import run_bass_kernel_spmd
from concourse.masks import make_identity
from concourse.tile import TileContext

B, S, H = 32, 2048, 1024
NCORES = 8
BL = B // NCORES          # batches per core
HC = H // 128             # 8 contraction chunks
OT = H // 128             # 8 output-feature tiles
NT = 512                  # matmul moving free dim
ST = S // NT              # 4 s-tiles per batch
SC = S // 128             # 16 s-chunks per batch (context contraction)

F32 = mybir.dt.float32
F32R = mybir.dt.float32r
BF16 = mybir.dt.bfloat16
AXX = mybir.AxisListType.X
TANH = mybir.ActivationFunctionType.Tanh
EXP = mybir.ActivationFunctionType.Exp

_compiled_nc = None


def _build():
    nc = bacc.Bacc("TRN2", target_bir_lowering=False, debug=False, num_devices=NCORES)

    encT_d = nc.dram_tensor("encT", [BL, H, S], F32R, kind="ExternalInput")
    enc_d = nc.dram_tensor("enc", [BL, S, H], BF16, kind="ExternalInput")
    wt_d = nc.dram_tensor("wt", [H, H], F32, kind="ExternalInput")       # W^T [h, o]
    hid_d = nc.dram_tensor("hid", [128, HC, BL], F32, kind="ExternalInput")
    vc_d = nc.dram_tensor("vc", [128, OT], F32, kind="ExternalInput")
    bc_d = nc.dram_tensor("bc", [128, OT], F32, kind="ExternalInput")
    ctx_d = nc.dram_tensor("ctx", [BL, H], F32, kind="ExternalOutput")
    attn_d = nc.dram_tensor("attn", [BL, S], F32, kind="ExternalOutput")

    with TileContext(nc) as tc:
        with (
            tc.tile_pool(name="singles", bufs=1) as singles,
            tc.tile_pool(name="xT", bufs=2) as xpool,
            tc.tile_pool(name="energy", bufs=3) as epool,
            tc.tile_pool(name="encn", bufs=24) as encpool,
            tc.tile_pool(name="smalls", bufs=4) as smalls,
            tc.tile_pool(name="attnp", bufs=2) as attnp,
            tc.tile_pool(name="acols", bufs=2) as acols,
            tc.tile_pool(name="ctxp", bufs=2) as ctxp,
            tc.tile_pool(name="ps_e", bufs=2, space="PSUM") as ps_e,
            tc.tile_pool(name="ps_sc", bufs=2, space="PSUM") as ps_sc,
            tc.tile_pool(name="ps_c", bufs=1, space="PSUM") as ps_c,
            tc.tile_pool(name="ps_b", bufs=1, space="PSUM") as ps_b,
            tc.tile_pool(name="ps_t", bufs=1, space="PSUM") as ps_t_pool,
        ):
            # ---- resident parameters -------------------------------------
            # Small params first (the bias matmuls need them immediately);
            # W^T per h-chunk so the first matmuls start a couple of us in
            # instead of waiting for the whole 4MB.
            hid_sb = singles.tile([128, HC, BL], F32R)
            nc.gpsimd.dma_start(out=hid_sb[:], in_=hid_d.ap())
            vc_sb = singles.tile([128, OT], F32R)
            nc.gpsimd.dma_start(out=vc_sb[:], in_=vc_d.ap())
            bc_sb = singles.tile([128, OT], F32)
            nc.sync.dma_start(out=bc_sb[:], in_=bc_d.ap())
            ident16 = singles.tile([SC, SC], F32)
            make_identity(nc, ident16[:])
            wt_sb = singles.tile([128, HC, H], F32R)     # [h_local, hc, o]
            wt_r = wt_d.ap().rearrange("(hc p) o -> hc p o", p=128)
            for hc in range(HC):
                nc.gpsimd.dma_start(out=wt_sb[:, hc, :], in_=wt_r[hc])

            # ---- per-(batch, o) bias: hidden @ W^T + b_attn --------------
            # Computed transposed ([batch, o], 16 wide matmuls) then flipped
            # with 8 small PE transposes — much cheaper than 64 narrow
            # matmuls, and consumes W^T chunks as their DMAs land.
            bias_sb = singles.tile([128, OT, BL], F32)
            biasT_sb = singles.tile([BL, H], F32)
            for half in range(2):
                bps = ps_b.tile([BL, NT], F32)
                for hc in range(HC):
                    nc.tensor.matmul(
                        bps[:],
                        hid_sb[:, hc, :],
                        wt_sb[:, hc, half * NT : (half + 1) * NT],
                        start=(hc == 0),
                        stop=(hc == HC - 1),
                    )
                nc.vector.tensor_copy(
                    out=biasT_sb[:, half * NT : (half + 1) * NT], in_=bps[:]
                )
            for ot in range(OT):
                tps = ps_t_pool.tile([128, BL], F32, tag="ps_t")
                nc.tensor.transpose(
                    out=tps[:],
                    in_=biasT_sb[:, ot * 128 : (ot + 1) * 128],
                    identity=ident16[0:BL, 0:BL],
                )
                nc.vector.tensor_scalar_add(bias_sb[:, ot, :], tps[:], bc_sb[:, ot : ot + 1])

            def emit_context(b, st16):
                """context[b] = attn[b] @ enc[b], contracting over S.

                Deferred by one batch so the in-order PE stream never stalls
                on batch b's softmax: by the time PE reaches these
                instructions it has already run batch b+1's GEMM.
                """
                ps_t = ps_t_pool.tile([128, SC], F32, tag="ps_t")
                nc.tensor.transpose(out=ps_t[:], in_=st16[:], identity=ident16[:])
                atc = acols.tile([128, SC], BF16)
                nc.vector.tensor_copy(out=atc[:], in_=ps_t[:])
                psc = ps_c.tile([1, H], F32)
                for sc in range(SC):
                    encn = encpool.tile([128, H], BF16)
                    nc.sync.dma_start(
                        out=encn[:], in_=enc_d.ap()[b, sc * 128 : (sc + 1) * 128, :]
                    )
                    nc.tensor.matmul(
                        psc[:, 0:NT], atc[:, sc : sc + 1], encn[:, 0:NT],
                        start=(sc == 0), stop=(sc == SC - 1),
                    )
                    nc.tensor.matmul(
                        psc[:, NT:H], atc[:, sc : sc + 1], encn[:, NT:H],
                        start=(sc == 0), stop=(sc == SC - 1),
                    )
                ctxs = ctxp.tile([1, H], F32)
                nc.vector.tensor_copy(out=ctxs[:], in_=psc[:])
                nc.sync.dma_start(out=ctx_d.ap()[b : b + 1, :], in_=ctxs[:])

            prev = None  # (batch_idx, atc tile) whose context emission is deferred
            for b in range(BL):
                # ---- energy GEMM + v-reduction -> scores -----------------
                # Each v-matmul is deferred by one ot-group (across s_tile
                # boundaries too) so the tanh it consumes finishes in the
                # shadow of the next group's 8 GEMM matmuls and the in-order
                # PE stream never stalls.
                scores_sb = attnp.tile([1, S], F32)
                pending_v = None   # ((args), (kwargs), scores_copy_or_None)
                pss_tiles = {}

                def flush_v():
                    nonlocal pending_v
                    if pending_v is not None:
                        args, kwargs, copy_args = pending_v
                        nc.tensor.matmul(*args, **kwargs)
                        if copy_args is not None:
                            nc.vector.tensor_copy(out=copy_args[0], in_=copy_args[1])
                        pending_v = None

                for st in range(ST):
                    xT = xpool.tile([128, HC, NT], F32R)
                    xT_src = encT_d.ap()[b].rearrange("(hc p) s -> p hc s", p=128)[
                        :, :, st * NT : (st + 1) * NT
                    ]
                    if b == 0 and st == 0:
                        # Per-chunk so the very first matmul starts after
                        # 256KB instead of 2MB.
                        for hc in range(HC):
                            nc.sync.dma_start(out=xT[:, hc, :], in_=xT_src[:, hc, :])
                    else:
                        nc.sync.dma_start(out=xT[:], in_=xT_src)
                    pss = ps_sc.tile([1, NT], F32)
                    pss_tiles[st] = pss
                    for ot in range(OT):
                        pse = ps_e.tile([128, NT], F32)
                        for hc in range(HC):
                            nc.tensor.matmul(
                                pse[:],
                                wt_sb[:, hc, ot * 128 : (ot + 1) * 128],
                                xT[:, hc, :],
                                start=(hc == 0),
                                stop=(hc == HC - 1),
                            )
                        en = epool.tile([128, NT], F32R)
                        nc.scalar.activation(
                            out=en[:], in_=pse[:], func=TANH,
                            bias=bias_sb[:, ot, b : b + 1], scale=1.0,
                        )
                        flush_v()
                        copy_args = None
                        if ot == OT - 1:
                            copy_args = (
                                scores_sb[:, st * NT : (st + 1) * NT],
                                pss[:],
                            )
                        pending_v = (
                            (pss[:], vc_sb[:, ot : ot + 1], en[:]),
                            dict(start=(ot == 0), stop=(ot == OT - 1)),
                            copy_args,
                        )
                flush_v()

                # ---- softmax over S (single-lane, in place) --------------
                negmax = smalls.tile([1, 1], F32)
                nc.vector.reduce_max(negmax[:], scores_sb[:], axis=AXX, negate=True)
                nc.scalar.activation(
                    out=scores_sb[:], in_=scores_sb[:], func=EXP, bias=negmax[:], scale=1.0
                )
                ssum = smalls.tile([1, 1], F32)
                nc.vector.reduce_sum(ssum[:], scores_sb[:], axis=AXX)
                rinv = smalls.tile([1, 1], F32)
                nc.vector.reciprocal(rinv[:], ssum[:])
                nc.vector.tensor_scalar_mul(scores_sb[:], scores_sb[:], rinv[:])
                nc.sync.dma_start(out=attn_d.ap()[b : b + 1, :], in_=scores_sb[:])

                # attn row reshaped to [16, 128] with one contiguous DMA; the
                # PE transpose to [128, 16] columns happens in emit_context.
                st16 = acols.tile([SC, 128], F32, tag="st16")
                nc.gpsimd.dma_start(out=st16[:], in_=scores_sb[0:1, :])

                if prev is not None:
                    emit_context(*prev)
                prev = (b, st16)

            emit_context(*prev)

    nc.compile()
    return nc


def _get_nc():
    global _compiled_nc
    if _compiled_nc is None:
        _compiled_nc = _build()
    return _compiled_nc


def kernel(hidden, encoder_outputs, W_attn, b_attn, v, _want_results=False, **run_kwargs):
    hidden = np.asarray(hidden, dtype=np.float32)
    enc = np.asarray(encoder_outputs, dtype=np.float32)
    W_attn = np.asarray(W_attn, dtype=np.float32)
    b_attn = np.asarray(b_attn, dtype=np.float32)
    v = np.asarray(v, dtype=np.float32)

    WT = np.ascontiguousarray(W_attn.T)                       # [h, o]
    vc = np.ascontiguousarray(v.reshape(OT, 128).T)           # [128, ot]
    bc = np.ascontiguousarray(b_attn.reshape(OT, 128).T)      # [128, ot]

    in_maps = []
    enc_bf16 = enc.astype(ml_dtypes.bfloat16)
    for i in range(NCORES):
        sl = slice(i * BL, (i + 1) * BL)
        enc_i = np.ascontiguousarray(enc_bf16[sl])            # [BL, S, H] bf16
        encT_i = np.ascontiguousarray(enc[sl].transpose(0, 2, 1))  # [BL, H, S]
        hid_i = np.ascontiguousarray(
            hidden[sl].reshape(BL, HC, 128).transpose(2, 1, 0)
        )                                                      # [128, hc, b]
        in_maps.append(
            {"encT": encT_i, "enc": enc_i, "wt": WT, "hid": hid_i, "vc": vc, "bc": bc}
        )

    nc = _get_nc()
    res = run_bass_kernel_spmd(nc, in_maps, core_ids=list(range(NCORES)), **run_kwargs)
    context = np.concatenate([res.results[i]["ctx"] for i in range(NCORES)], axis=0)
    attn = np.concatenate([res.results[i]["attn"] for i in range(NCORES)], axis=0)
    if _want_results:
        return (context, attn), res
    return (context, attn)


if __name__ == "__main__":
    rng = np.random.default_rng(0)
    out = kernel(
        hidden=rng.standard_normal((B, H), dtype=np.float32),
        encoder_outputs=rng.standard_normal((B, S, H), dtype=np.float32),
        W_attn=rng.standard_normal((H, H), dtype=np.float32) / 32.0,
        b_attn=rng.standard_normal(H, dtype=np.float32) * 0.01,
        v=rng.random(H, dtype=np.float32),
    )
    print("context", out[0].shape, "attn", out[1].shape)
